# revision 22
# baseline (speedup 1.0000x reference)
"""Trainium2 Bass kernel: single-head self-attention with residual.

Reference computation (per batch b):
    q = x @ Wq + bq ; k = x @ Wk + bk ; v = x @ Wv + bv
    scores = q @ k^T / sqrt(U) ; attn = softmax(scores, axis=-1)
    out = x + (attn @ v) @ Wp + bp

Shapes: x [B=4, N=4096, U=512], weights [512, 512], biases [512].

Sharding: 8 cores = 4 batches x 2 sequence halves. Core i owns batch
b = i // 2, Q-rows h = i % 2 (2048 rows). Each core receives its
batch's FULL x (host-side replication plays the role of the K/V
all-gather), so there are no on-device collectives and cores are fully
independent.

Weight folding (the big win over a direct port): softmax is invariant
to per-row shifts, so with zero bq the scores can be computed as
    S = x (Wq Wk^T) x^T  = x M x^T
and the output projection as
    (A x Wv) Wp = A x (Wv Wp) = A x W2
with M = Wq Wk^T and W2 = Wv Wp folded ON THE HOST (weight-only
preprocessing, like fusing BN into a conv). This removes the Q, K and
V projections entirely: the device computes only q' = xq M over its
OWN NQ rows (half the cost of a K-side fold), scores against the
already-resident raw x^T, PV against raw x, and the W2 output
projection.

Bias handling stays exact: bk adds a per-Q-row constant to scores
(softmax drops it); bv/bp fold into the residual (attn rows sum to 1:
bconst = bv @ Wp + bp); a nonzero bq adds the per-KEY column term
c_j = x_j . (Wk bq) + bq.bk which is folded into the exp bias via a
separate build variant (never triggered by this problem's zero-filled
biases).

Device layout choices (inherited from the tuned direct port):
  - All matmuls run in fp8e4 (TRN E4M3) with perf_mode=DoubleRow:
    operands carry [128, 2, *] APs so each instruction contracts 256
    (~213 ns per 512-wide MM at the 2x fp8 peak rate).
  - Scales: M/W2 are scaled by WS=16 on the host; scores come out 16x,
    folded into the softmax exp scale; exp gets a -SHIFT bias; ctx is
    written at CTX_SCL=1/2; the den 'ones' stationary is 8.0 so one
    reciprocal absorbs 16*CTX_SCL = 8.
  - x arrives in three host-prepped forms: xT [U, N] fp8 (score
    stationary over all keys + the q'-projection moving operand), xn
    [N, U] fp8 (PV stationary), xq [NQ, U] bf16 (residual + folded
    bconst). Keys are ordered [own half, other half] so the Q slice is
    always columns 0..NQ of xT (one SPMD graph for all cores); softmax
    is permutation-invariant since xn shares the ordering.
  - Scores are computed transposed (S^T [nk, nq] tiles), exp'd on the
    Scalar engine straight out of PSUM, consumed as the moving operand
    of the PV matmul. The softmax denominator is a DoubleRow
    ones-matmul over the ex tiles, issued BEFORE the last PV block so
    the reciprocal is ready when the output projection drains.
  - ctxT casts alternate Scalar/Vector so neither engine serializes
    the tail; score PSUM tiles are paired ([128, 2, 512], one exp per
    pair); the output projection shares the ctx PSUM ring.
"""

import numpy as np

B, N, U = 4, 4096, 512
NCORES = 8
NQ = N // 2          # 2048 Q rows per core
P = 128              # partitions
C = U // P           # 4 u-chunks
NKC = N // P         # 32 nk chunks
NKT = N // 512       # 8 nk 512-tiles
NQT = NQ // 512      # 4 nq 512-tiles
NQC = NQ // P        # 16 q-row chunks of 128
SCALE = float(1.0 / np.sqrt(np.float32(U)))
WS = 16.0            # host-side weight scale (keeps M/W2 out of fp8 subnormals)
SHIFT = 3.0          # softmax shift: ex = exp(s - SHIFT)
CTX_SCL = 1.0 / 2.0  # ctx psum -> fp8 scale
# exp input: psum = (16*a).(x) = 16*S_raw  ->  scale = SCALE/16
SCALE_EFF = SCALE / WS
# out = pj * recip + xq needs recip = 1/(WS*CTX_SCL*den) = 1/(8*den):
# the denominator 'ones' stationary is DEN_W so reciprocal(DEN_W*den) works.
DEN_W = WS * CTX_SCL

_CACHE = {}


def warm_ps_out(t):
    return t[:]


def _build_nc(has_c=False):
    from concourse import bacc, mybir, tile

    f32 = mybir.dt.float32
    bf16 = mybir.dt.bfloat16
    f8 = mybir.dt.float8e4
    Copy = mybir.ActivationFunctionType.Copy
    Exp = mybir.ActivationFunctionType.Exp
    Mult = mybir.AluOpType.mult
    Add = mybir.AluOpType.add
    DR = mybir.MatmulPerfMode.DoubleRow

    nc = bacc.Bacc("TRN2", target_bir_lowering=False, debug=False, num_devices=NCORES)

    xT_d = nc.dram_tensor("xT", [U, N], f8, kind="ExternalInput")
    xn_d = nc.dram_tensor("xn", [N, U], f8, kind="ExternalInput")
    xq_d = nc.dram_tensor("xq", [NQ, U], bf16, kind="ExternalInput")
    Wm_d = nc.dram_tensor("Wm", [U, U], f8, kind="ExternalInput")
    W2_d = nc.dram_tensor("W2", [U, U], f8, kind="ExternalInput")
    if has_c:
        cb_d = nc.dram_tensor("cb", [N], f32, kind="ExternalInput")
    out_d = nc.dram_tensor("out", [NQ, U], bf16, kind="ExternalOutput")

    with tile.TileContext(nc) as tc:
        with (
            tc.tile_pool(name="big", bufs=1) as big,
            tc.tile_pool(name="small", bufs=1) as small,
            tc.tile_pool(name="w3", bufs=1) as w3,
            tc.tile_pool(name="st_ps", bufs=2, space="PSUM") as st_ps,
            tc.tile_pool(name="ctx_ps", bufs=4, space="PSUM") as ctx_ps,
        ):
            # ---- persistent tensors -------------------------------------
            xT_sb = big.tile([P, C, N], f8, tag="xT_sb")      # x^T [u, n]
            qpT = big.tile([P, C, NQ], f8, tag="qpT")         # 16*(M^T xq^T)
            xn_sb = big.tile([P, NKC, U], f8, tag="xn_sb")    # x [nk, u]
            xq_sb = big.tile([P, NQC, U], bf16, tag="xq_sb")  # residual+bconst

            warm = w3.tile([P, 512], bf16, tag="warm")
            nc.gpsimd.memset(warm[:], 0.0)
            ones2 = small.tile([P, 2, 16], f8, tag="ones2")
            nc.vector.memset(ones2[:], DEN_W)
            negshift = small.tile([P, 1], f32, tag="negshift")
            nc.vector.memset(negshift[:], -SHIFT)
            one_one = small.tile([1, 1], f32, tag="one_one")
            nc.vector.memset(one_one[:], 1.0)
            # dummy exp: loads the Exp activation table while the kernel is
            # still waiting on its first DMAs (saves ~0.4us off the first
            # real exp on the critical phase A->B transition)
            exp_warm = small.tile([P, 1], f32, tag="exp_warm")
            nc.scalar.activation(exp_warm[:], negshift[:], Exp)
            if has_c:
                cb_sb = small.tile([P, NKC], f32, tag="cb")
                nc.sync.dma_start(
                    cb_sb[:], cb_d.ap().rearrange("(k p) -> p k", p=P))

            xT_r = xT_d.ap().rearrange("(c p) n -> p c n", p=P)
            wm_r = Wm_d.ap().rearrange("(c p) n -> p c n", p=P)

            # ---- phase A DMAs --------------------------------------------
            # DMA *issue* costs ~600ns per descriptor on the issuing engine,
            # so use one coalesced 3D DMA per xT tile, all on the sync queue
            # (the gpsimd/scalar rings start their transfers too late for the
            # phase-A critical path).
            wm = w3.tile([P, C, U], f8, tag="wm")
            nc.sync.dma_start(wm[:, 0:2, :], wm_r[:, 0:2, :])
            nc.sync.dma_start(xT_sb[:, 0:2, 0:512], xT_r[:, 0:2, 0:512])
            nc.sync.dma_start(wm[:, 2:4, :], wm_r[:, 2:4, :])
            nc.sync.dma_start(xT_sb[:, 2:4, 0:512], xT_r[:, 2:4, 0:512])
            for t in range(1, NKT):
                nc.sync.dma_start(
                    xT_sb[:, :, t * 512:(t + 1) * 512],
                    xT_r[:, :, t * 512:(t + 1) * 512])
            # PV stationary / residual / out-proj weights behind wm on gpsimd
            xn_r = xn_d.ap().rearrange("(k p) u -> p k u", p=P)
            for g in range(4):
                nc.sync.dma_start(
                    xn_sb[:, g * 8:(g + 1) * 8, :], xn_r[:, g * 8:(g + 1) * 8, :])
            wp = w3.tile([P, C, U], f8, tag="wp")
            nc.sync.dma_start(wp[:], W2_d.ap().rearrange("(c p) n -> p c n", p=P))
            nc.sync.dma_start(
                xq_sb[:], xq_d.ap().rearrange("(c p) u -> p c u", p=P))

            # keep the PE hot while the first wm/xT chunks stream in
            wps = st_ps.tile([P, 512], f32, tag="st", name="warm_ps")
            for i in range(8):
                nc.tensor.matmul(
                    warm_ps_out(wps), warm[:, 0:P], warm[:],
                    start=(i == 0), stop=(i == 7),
                )
            nc.scalar.copy(warm[:, 0:4], wps[:, 0:4])  # retire warm psum

            # ---- phase B pools (opened early: the t=0 softmax pipeline is
            # prerolled inside phase A) ----------------------------------
            with (
                tc.tile_pool(name="expp", bufs=2) as expp,
                tc.tile_pool(name="ctxp", bufs=2) as ctxp,
                tc.tile_pool(name="io", bufs=6) as iop,
            ):
                # phase A: q'^T = M^T xq^T over only this core's NQ rows
                # (folding M into the Q side halves the projection work vs
                # the K side: 32 DR matmuls, and the score stationary is
                # the already-resident raw x^T)
                ex0 = None
                for t in range(NQT):
                    if t == 0:
                        # c2-outer for the first tile: the first 4 matmuls
                        # depend only on the first halves of the wm/xT DMAs
                        pss = [ctx_ps.tile([P, 512], f32, tag="ctx",
                                           name=f"qp0_{m}") for m in range(C)]
                        for c2 in range(2):
                            for m in range(C):
                                nc.tensor.matmul(
                                    pss[m][:],
                                    wm[:, 2 * c2:2 * c2 + 2, m * P:(m + 1) * P],
                                    xT_sb[:, 2 * c2:2 * c2 + 2, 0:512],
                                    start=(c2 == 0), stop=(c2 == 1),
                                    perf_mode=DR,
                                )
                        for m in range(C):
                            if m % 2 == 0:
                                nc.scalar.copy(qpT[:, m, 0:512], pss[m][:])
                            else:
                                nc.vector.tensor_copy(qpT[:, m, 0:512],
                                                      pss[m][:])
                    else:
                        for m in range(C):
                            ps = ctx_ps.tile([P, 512], f32, tag="ctx")
                            for c2 in range(2):
                                nc.tensor.matmul(
                                    ps[:],
                                    wm[:, 2 * c2:2 * c2 + 2, m * P:(m + 1) * P],
                                    xT_sb[:, 2 * c2:2 * c2 + 2,
                                          t * 512:(t + 1) * 512],
                                    start=(c2 == 0), stop=(c2 == 1),
                                    perf_mode=DR,
                                )
                            if m % 2 == 0:
                                nc.scalar.copy(qpT[:, m, t * 512:(t + 1) * 512],
                                               ps[:])
                            else:
                                nc.vector.tensor_copy(
                                    qpT[:, m, t * 512:(t + 1) * 512], ps[:])
                    if t == 0:
                        # preroll the first score block of phase B (t=0,
                        # kk=0: key chunks 0-3, within xT tile 0) so the
                        # exps run on the idle scalar engine during phase A
                        # and PV can start the moment phase A ends.
                        ex0 = expp.tile([P, NKC, 512], f8, tag="ex",
                                        name="ex_t0")
                        for nk in range(0, 4, 2):
                            st = st_ps.tile([P, 2, 512], f32, tag="st")
                            for j in range(2):
                                for c2 in range(2):
                                    nc.tensor.matmul(
                                        st[:, j, :],
                                        xT_sb[:, 2 * c2:2 * c2 + 2,
                                              (nk + j) * P:(nk + j + 1) * P],
                                        qpT[:, 2 * c2:2 * c2 + 2, 0:512],
                                        start=(c2 == 0), stop=(c2 == 1),
                                        perf_mode=DR,
                                    )
                            if not has_c:
                                nc.scalar.activation(
                                    ex0[:, nk:nk + 2, :], st[:], Exp,
                                    scale=SCALE_EFF, bias=negshift[:])
                            else:
                                for j in range(2):
                                    nc.scalar.activation(
                                        ex0[:, nk + j, :], st[:, j, :], Exp,
                                        scale=SCALE_EFF,
                                        bias=cb_sb[:, nk + j:nk + j + 1])

                # ---- phase B: attention + projection --------------------
                for t in range(NQT):
                    nq_sl = slice(t * 512, (t + 1) * 512)
                    ctx_psums = [
                        ctx_ps.tile([P, 512], f32, tag="ctx", name=f"ctx_{t}_{u}")
                        for u in range(C)
                    ]

                    if t == 0:
                        ex = ex0
                        # kk=0 scores/exps were prerolled into phase A:
                        # drain their PV block immediately
                        for k2 in range(2):
                            for u in range(C):
                                nc.tensor.matmul(
                                    ctx_psums[u][:],
                                    xn_sb[:, 2 * k2:2 * k2 + 2,
                                          u * P:(u + 1) * P],
                                    ex[:, 2 * k2:2 * k2 + 2, :],
                                    start=(k2 == 0), stop=False,
                                    perf_mode=DR,
                                )
                        kk_start = 4
                    else:
                        ex = expp.tile([P, NKC, 512], f8, tag="ex")
                        kk_start = 0
                    den = None
                    for kk in range(kk_start, NKC, 4):
                        for nk in range(kk, kk + 4, 2):
                            # paired score tiles: one 2-bank PSUM tile, one
                            # [128, 2, 512] exp per two key chunks
                            st = st_ps.tile([P, 2, 512], f32, tag="st")
                            for j in range(2):
                                for c2 in range(2):
                                    nc.tensor.matmul(
                                        st[:, j, :],
                                        xT_sb[:, 2 * c2:2 * c2 + 2,
                                              (nk + j) * P:(nk + j + 1) * P],
                                        qpT[:, 2 * c2:2 * c2 + 2, nq_sl],
                                        start=(c2 == 0), stop=(c2 == 1),
                                        perf_mode=DR,
                                    )
                            if not has_c:
                                nc.scalar.activation(
                                    ex[:, nk:nk + 2, :], st[:], Exp,
                                    scale=SCALE_EFF, bias=negshift[:])
                            else:
                                for j in range(2):
                                    nc.scalar.activation(
                                        ex[:, nk + j, :], st[:, j, :], Exp,
                                        scale=SCALE_EFF,
                                        bias=cb_sb[:, nk + j:nk + j + 1])
                        if kk == NKC - 4:
                            # softmax denominator before the last PV block:
                            # recip computes during the final PV chains, so
                            # the tail drain is just casts + out-proj.
                            den = st_ps.tile([P, 2, 512], f32, tag="st",
                                             name=f"den_{t}")
                            for k2 in range(NKC // 2):
                                nc.tensor.matmul(
                                    den[0:1, 0, :], ones2[:, 0:2, 0:1],
                                    ex[:, 2 * k2:2 * k2 + 2, :],
                                    start=(k2 == 0), stop=(k2 == NKC // 2 - 1),
                                    perf_mode=DR,
                                )
                            den_row = small.tile([1, 512], f32, tag="den_row",
                                                 bufs=2)
                            nc.scalar.copy(den_row[:], den[0:1, 0, :])
                        for k2 in range(kk // 2, kk // 2 + 2):
                            for u in range(C):
                                nc.tensor.matmul(
                                    ctx_psums[u][:],
                                    xn_sb[:, 2 * k2:2 * k2 + 2,
                                          u * P:(u + 1) * P],
                                    ex[:, 2 * k2:2 * k2 + 2, :],
                                    start=(k2 == 0), stop=(k2 == NKC // 2 - 1),
                                    perf_mode=DR,
                                )
                    for s in range(4):
                        # K=1 matmul == transpose of a 128-wide row slice
                        nc.tensor.matmul(
                            den[:, 1, s:s + 1], den_row[0:1, s * P:(s + 1) * P],
                            one_one[:], start=True, stop=True,
                        )
                    recip = small.tile([P, C], f32, tag="recip", bufs=2)
                    nc.vector.reciprocal(recip[:], den[:, 1, 0:4])

                    # ctx psum -> fp8, alternating engines so the tail isn't
                    # serialized on one of them
                    ctxT = ctxp.tile([P, C, 512], f8, tag="ctxT")
                    for u in range(C):
                        if u % 2 == 0:
                            nc.scalar.activation(
                                ctxT[:, u, :], ctx_psums[u][:], Copy,
                                scale=CTX_SCL)
                        else:
                            nc.vector.tensor_scalar_mul(
                                ctxT[:, u, :], ctx_psums[u][:], CTX_SCL)

                    for s in range(4):  # nq sub-chunks of 128
                        pj = ctx_ps.tile([P, 512], f32, tag="ctx",
                                         name=f"pj_{t}_{s}")
                        for c2 in range(2):
                            nc.tensor.matmul(
                                pj[:],
                                ctxT[:, 2 * c2:2 * c2 + 2, s * P:(s + 1) * P],
                                wp[:, 2 * c2:2 * c2 + 2, :],
                                start=(c2 == 0), stop=(c2 == 1),
                                perf_mode=DR,
                            )
                        r0 = t * 4 + s
                        o = iop.tile([P, U], bf16, tag="o")
                        # o = pj * recip (per-partition) + xq; the last tile
                        # computes in halves so the first half's writeback
                        # overlaps the second half's fusion
                        dma_eng = nc.sync if s % 2 == 0 else nc.scalar
                        if t == NQT - 1 and s >= 2:
                            for hh in range(2):
                                hs = slice(hh * (U // 2), (hh + 1) * (U // 2))
                                nc.vector.scalar_tensor_tensor(
                                    o[:, hs], pj[:, hs], recip[:, s:s + 1],
                                    xq_sb[:, r0, hs], Mult, Add,
                                )
                                dma_eng.dma_start(
                                    out_d[r0 * P:(r0 + 1) * P, hs], o[:, hs])
                        else:
                            nc.vector.scalar_tensor_tensor(
                                o[:], pj[:], recip[:, s:s + 1], xq_sb[:, r0, :],
                                Mult, Add,
                            )
                            dma_eng.dma_start(
                                out_d[r0 * P:(r0 + 1) * P, :], o[:])

    nc.compile()
    return nc


def _get_nc(has_c=False):
    key = ("nc", has_c)
    if key not in _CACHE:
        _CACHE[key] = _build_nc(has_c)
    return _CACHE[key]


def make_in_maps(x, Wq, bq, Wk, bk, Wv, bv, Wp, bp):
    x = np.asarray(x, np.float32)
    Wq = np.asarray(Wq, np.float32)
    bq = np.asarray(bq, np.float32)
    Wk = np.asarray(Wk, np.float32)
    bk = np.asarray(bk, np.float32)
    Wv = np.asarray(Wv, np.float32)
    bv = np.asarray(bv, np.float32)
    Wp = np.asarray(Wp, np.float32)
    bp = np.asarray(bp, np.float32)

    # folded weights: S = x M x^T with M = Wq Wk^T, folded into the Q side
    # (device computes q' = xq M over only its own NQ rows); out-proj
    # weight W2 = Wv Wp
    Wm = Wq @ Wk.T
    W2 = Wv @ Wp
    # attn rows sum to 1 => bv contributes bv @ Wp to every output row
    bconst = (bv @ Wp + bp).astype(np.float32)
    # nonzero bq adds the per-key column term c_j = x_j.(Wk bq) + bq.bk
    has_c = bool(np.any(bq != 0.0))

    import ml_dtypes
    f8 = ml_dtypes.float8_e4m3
    bf16 = ml_dtypes.bfloat16

    def q8(a):
        return np.ascontiguousarray(np.clip(a, -240.0, 240.0)).astype(f8)

    Wm8, W28 = q8(Wm * WS), q8(W2 * WS)

    in_maps = []
    for core in range(NCORES):
        b, h = core // 2, core % 2
        xb = np.ascontiguousarray(x[b])                       # [N, U]
        # keys ordered [own half, other half]; Q slice = first NQ columns
        perm = np.r_[h * NQ:(h + 1) * NQ, (1 - h) * NQ:(2 - h) * NQ]
        im = {
            "xT": q8(xb.T[:, perm]),
            "xn": q8(xb[perm]),
            "xq": np.ascontiguousarray(
                xb[h * NQ:(h + 1) * NQ] + bconst[None, :]).astype(bf16),
            "Wm": Wm8, "W2": W28,
        }
        if has_c:
            cvec = xb[perm] @ (Wk @ bq) + float(bq @ bk)
            im["cb"] = (cvec * 1.0 - SHIFT).astype(np.float32)
        in_maps.append(im)
    return in_maps, has_c


def gather_out(results):
    out = np.empty((B, N, U), np.float32)
    for core in range(NCORES):
        b, h = core // 2, core % 2
        out[b, h * NQ:(h + 1) * NQ] = np.asarray(
            results[core]["out"], dtype=np.float32)
    return out


def kernel(x, Wq, bq, Wk, bk, Wv, bv, Wp, bp):
    from concourse.bass_utils import run_bass_kernel_spmd

    in_maps, has_c = make_in_maps(x, Wq, bq, Wk, bk, Wv, bv, Wp, bp)
    nc = _get_nc(has_c)
    res = run_bass_kernel_spmd(nc, in_maps, core_ids=list(range(NCORES)))
    return gather_out(res.results)


# revision 23
# speedup vs baseline: 1.0063x; 1.0063x over previous
"""Trainium2 Bass kernel: single-head self-attention with residual.

Reference computation (per batch b):
    q = x @ Wq + bq ; k = x @ Wk + bk ; v = x @ Wv + bv
    scores = q @ k^T / sqrt(U) ; attn = softmax(scores, axis=-1)
    out = x + (attn @ v) @ Wp + bp

Shapes: x [B=4, N=4096, U=512], weights [512, 512], biases [512].

Sharding: 8 cores = 4 batches x 2 sequence halves. Core i owns batch
b = i // 2, Q-rows h = i % 2 (2048 rows). Each core receives its
batch's FULL x (host-side replication plays the role of the K/V
all-gather), so there are no on-device collectives and cores are fully
independent.

Weight folding (the big win over a direct port): softmax is invariant
to per-row shifts, so with zero bq the scores can be computed as
    S = x (Wq Wk^T) x^T  = x M x^T
and the output projection as
    (A x Wv) Wp = A x (Wv Wp) = A x W2
with M = Wq Wk^T and W2 = Wv Wp folded ON THE HOST (weight-only
preprocessing, like fusing BN into a conv). This removes the Q, K and
V projections entirely: the device computes only q' = xq M over its
OWN NQ rows (half the cost of a K-side fold), scores against the
already-resident raw x^T, PV against raw x, and the W2 output
projection.

Bias handling stays exact: bk adds a per-Q-row constant to scores
(softmax drops it); bv/bp fold into the residual (attn rows sum to 1:
bconst = bv @ Wp + bp); a nonzero bq adds the per-KEY column term
c_j = x_j . (Wk bq) + bq.bk which is folded into the exp bias via a
separate build variant (never triggered by this problem's zero-filled
biases).

Device layout choices (inherited from the tuned direct port):
  - All matmuls run in fp8e4 (TRN E4M3) with perf_mode=DoubleRow:
    operands carry [128, 2, *] APs so each instruction contracts 256
    (~213 ns per 512-wide MM at the 2x fp8 peak rate).
  - Scales: M/W2 are scaled by WS=16 on the host; scores come out 16x,
    folded into the softmax exp scale; exp gets a -SHIFT bias; ctx is
    written at CTX_SCL=1/2; the den 'ones' stationary is 8.0 so one
    reciprocal absorbs 16*CTX_SCL = 8.
  - x arrives in three host-prepped forms: xT [U, N] fp8 (score
    stationary over all keys + the q'-projection moving operand), xn
    [N, U] fp8 (PV stationary), xq [NQ, U] bf16 (residual + folded
    bconst). Keys are ordered [own half, other half] so the Q slice is
    always columns 0..NQ of xT (one SPMD graph for all cores); softmax
    is permutation-invariant since xn shares the ordering.
  - Scores are computed transposed (S^T [nk, nq] tiles), exp'd on the
    Scalar engine straight out of PSUM, consumed as the moving operand
    of the PV matmul. The softmax denominator is a DoubleRow
    ones-matmul over the ex tiles, issued BEFORE the last PV block so
    the reciprocal is ready when the output projection drains.
  - ctxT casts alternate Scalar/Vector so neither engine serializes
    the tail; score PSUM tiles are paired ([128, 2, 512], one exp per
    pair); the output projection shares the ctx PSUM ring.
"""

import numpy as np

B, N, U = 4, 4096, 512
NCORES = 8
NQ = N // 2          # 2048 Q rows per core
P = 128              # partitions
C = U // P           # 4 u-chunks
NKC = N // P         # 32 nk chunks
NKT = N // 512       # 8 nk 512-tiles
NQT = NQ // 512      # 4 nq 512-tiles
NQC = NQ // P        # 16 q-row chunks of 128
SCALE = float(1.0 / np.sqrt(np.float32(U)))
WS = 16.0            # host-side weight scale (keeps M/W2 out of fp8 subnormals)
SHIFT = 3.0          # softmax shift: ex = exp(s - SHIFT)
CTX_SCL = 1.0 / 2.0  # ctx psum -> fp8 scale
# exp input: psum = (16*a).(x) = 16*S_raw  ->  scale = SCALE/16
SCALE_EFF = SCALE / WS
# out = pj * recip + xq needs recip = 1/(WS*CTX_SCL*den) = 1/(8*den):
# the denominator 'ones' stationary is DEN_W so reciprocal(DEN_W*den) works.
DEN_W = WS * CTX_SCL

_CACHE = {}


def warm_ps_out(t):
    return t[:]


def _build_nc(has_c=False):
    from concourse import bacc, mybir, tile

    f32 = mybir.dt.float32
    bf16 = mybir.dt.bfloat16
    f8 = mybir.dt.float8e4
    Copy = mybir.ActivationFunctionType.Copy
    Exp = mybir.ActivationFunctionType.Exp
    Mult = mybir.AluOpType.mult
    Add = mybir.AluOpType.add
    DR = mybir.MatmulPerfMode.DoubleRow

    nc = bacc.Bacc("TRN2", target_bir_lowering=False, debug=False, num_devices=NCORES)

    xT_d = nc.dram_tensor("xT", [U, N], f8, kind="ExternalInput")
    xn_d = nc.dram_tensor("xn", [N, U], f8, kind="ExternalInput")
    xq_d = nc.dram_tensor("xq", [NQ, U], bf16, kind="ExternalInput")
    Wm_d = nc.dram_tensor("Wm", [U, U], f8, kind="ExternalInput")
    W2_d = nc.dram_tensor("W2", [U, U], f8, kind="ExternalInput")
    if has_c:
        cb_d = nc.dram_tensor("cb", [N], f32, kind="ExternalInput")
    out_d = nc.dram_tensor("out", [NQ, U], bf16, kind="ExternalOutput")

    with tile.TileContext(nc) as tc:
        with (
            tc.tile_pool(name="big", bufs=1) as big,
            tc.tile_pool(name="small", bufs=1) as small,
            tc.tile_pool(name="w3", bufs=1) as w3,
            tc.tile_pool(name="st_ps", bufs=2, space="PSUM") as st_ps,
            tc.tile_pool(name="ctx_ps", bufs=4, space="PSUM") as ctx_ps,
        ):
            # ---- persistent tensors -------------------------------------
            xT_sb = big.tile([P, C, N], f8, tag="xT_sb")      # x^T [u, n]
            qpT = big.tile([P, C, NQ], f8, tag="qpT")         # 16*(M^T xq^T)
            xn_sb = big.tile([P, NKC, U], f8, tag="xn_sb")    # x [nk, u]
            xq_sb = big.tile([P, NQC, U], bf16, tag="xq_sb")  # residual+bconst

            warm = w3.tile([P, 512], bf16, tag="warm")
            nc.gpsimd.memset(warm[:], 0.0)
            ones2 = small.tile([P, 2, 16], f8, tag="ones2")
            nc.vector.memset(ones2[:], DEN_W)
            negshift = small.tile([P, 1], f32, tag="negshift")
            nc.vector.memset(negshift[:], -SHIFT)
            one_one = small.tile([1, 1], f32, tag="one_one")
            nc.vector.memset(one_one[:], 1.0)
            # dummy exp: loads the Exp activation table while the kernel is
            # still waiting on its first DMAs (saves ~0.4us off the first
            # real exp on the critical phase A->B transition)
            exp_warm = small.tile([P, 1], f32, tag="exp_warm")
            nc.scalar.activation(exp_warm[:], negshift[:], Exp)
            if has_c:
                cb_sb = small.tile([P, NKC], f32, tag="cb")
                nc.sync.dma_start(
                    cb_sb[:], cb_d.ap().rearrange("(k p) -> p k", p=P))

            xT_r = xT_d.ap().rearrange("(c p) n -> p c n", p=P)
            wm_r = Wm_d.ap().rearrange("(c p) n -> p c n", p=P)

            # ---- phase A DMAs --------------------------------------------
            # DMA *issue* costs ~600ns per descriptor on the issuing engine,
            # so use one coalesced 3D DMA per xT tile, all on the sync queue
            # (the gpsimd/scalar rings start their transfers too late for the
            # phase-A critical path).
            wm = w3.tile([P, C, U], f8, tag="wm")
            nc.sync.dma_start(wm[:], wm_r[:])
            for t in range(NKT):
                nc.sync.dma_start(
                    xT_sb[:, :, t * 512:(t + 1) * 512],
                    xT_r[:, :, t * 512:(t + 1) * 512])
            # PV stationary / residual / out-proj weights behind wm on gpsimd
            xn_r = xn_d.ap().rearrange("(k p) u -> p k u", p=P)
            for g in range(4):
                nc.sync.dma_start(
                    xn_sb[:, g * 8:(g + 1) * 8, :], xn_r[:, g * 8:(g + 1) * 8, :])
            wp = w3.tile([P, C, U], f8, tag="wp")
            nc.sync.dma_start(wp[:], W2_d.ap().rearrange("(c p) n -> p c n", p=P))
            nc.sync.dma_start(
                xq_sb[:], xq_d.ap().rearrange("(c p) u -> p c u", p=P))

            # keep the PE hot while the first wm/xT chunks stream in
            wps = st_ps.tile([P, 512], f32, tag="st", name="warm_ps")
            for i in range(11):
                nc.tensor.matmul(
                    warm_ps_out(wps), warm[:, 0:P], warm[:],
                    start=(i == 0), stop=(i == 10),
                )
            nc.scalar.copy(warm[:, 0:4], wps[:, 0:4])  # retire warm psum

            # ---- phase B pools (opened early: the t=0 softmax pipeline is
            # prerolled inside phase A) ----------------------------------
            with (
                tc.tile_pool(name="expp", bufs=2) as expp,
                tc.tile_pool(name="ctxp", bufs=2) as ctxp,
                tc.tile_pool(name="io", bufs=6) as iop,
            ):
                # phase A: q'^T = M^T xq^T over only this core's NQ rows
                # (folding M into the Q side halves the projection work vs
                # the K side: 32 DR matmuls, and the score stationary is
                # the already-resident raw x^T)
                ex0 = None
                for t in range(NQT):
                    for m in range(C):
                        ps = ctx_ps.tile([P, 512], f32, tag="ctx")
                        for c2 in range(2):
                            nc.tensor.matmul(
                                ps[:],
                                wm[:, 2 * c2:2 * c2 + 2, m * P:(m + 1) * P],
                                xT_sb[:, 2 * c2:2 * c2 + 2,
                                      t * 512:(t + 1) * 512],
                                start=(c2 == 0), stop=(c2 == 1),
                                perf_mode=DR,
                            )
                        if m % 2 == 0:
                            nc.scalar.copy(qpT[:, m, t * 512:(t + 1) * 512],
                                           ps[:])
                        else:
                            nc.vector.tensor_copy(
                                qpT[:, m, t * 512:(t + 1) * 512], ps[:])
                    if t == 0:
                        # preroll the first score block of phase B (t=0,
                        # kk=0: key chunks 0-3, within xT tile 0) so the
                        # exps run on the idle scalar engine during phase A
                        # and PV can start the moment phase A ends.
                        ex0 = expp.tile([P, NKC, 512], f8, tag="ex",
                                        name="ex_t0")
                        for nk in range(0, 4, 2):
                            st = st_ps.tile([P, 2, 512], f32, tag="st")
                            for j in range(2):
                                for c2 in range(2):
                                    nc.tensor.matmul(
                                        st[:, j, :],
                                        xT_sb[:, 2 * c2:2 * c2 + 2,
                                              (nk + j) * P:(nk + j + 1) * P],
                                        qpT[:, 2 * c2:2 * c2 + 2, 0:512],
                                        start=(c2 == 0), stop=(c2 == 1),
                                        perf_mode=DR,
                                    )
                            if not has_c:
                                nc.scalar.activation(
                                    ex0[:, nk:nk + 2, :], st[:], Exp,
                                    scale=SCALE_EFF, bias=negshift[:])
                            else:
                                for j in range(2):
                                    nc.scalar.activation(
                                        ex0[:, nk + j, :], st[:, j, :], Exp,
                                        scale=SCALE_EFF,
                                        bias=cb_sb[:, nk + j:nk + j + 1])

                # ---- phase B: attention + projection --------------------
                for t in range(NQT):
                    nq_sl = slice(t * 512, (t + 1) * 512)
                    ctx_psums = [
                        ctx_ps.tile([P, 512], f32, tag="ctx", name=f"ctx_{t}_{u}")
                        for u in range(C)
                    ]

                    if t == 0:
                        ex = ex0
                        # kk=0 scores/exps were prerolled into phase A:
                        # drain their PV block immediately
                        for k2 in range(2):
                            for u in range(C):
                                nc.tensor.matmul(
                                    ctx_psums[u][:],
                                    xn_sb[:, 2 * k2:2 * k2 + 2,
                                          u * P:(u + 1) * P],
                                    ex[:, 2 * k2:2 * k2 + 2, :],
                                    start=(k2 == 0), stop=False,
                                    perf_mode=DR,
                                )
                        kk_start = 4
                    else:
                        ex = expp.tile([P, NKC, 512], f8, tag="ex")
                        kk_start = 0
                    den = None
                    for kk in range(kk_start, NKC, 4):
                        for nk in range(kk, kk + 4, 2):
                            # paired score tiles: one 2-bank PSUM tile, one
                            # [128, 2, 512] exp per two key chunks
                            st = st_ps.tile([P, 2, 512], f32, tag="st")
                            for j in range(2):
                                for c2 in range(2):
                                    nc.tensor.matmul(
                                        st[:, j, :],
                                        xT_sb[:, 2 * c2:2 * c2 + 2,
                                              (nk + j) * P:(nk + j + 1) * P],
                                        qpT[:, 2 * c2:2 * c2 + 2, nq_sl],
                                        start=(c2 == 0), stop=(c2 == 1),
                                        perf_mode=DR,
                                    )
                            if not has_c:
                                nc.scalar.activation(
                                    ex[:, nk:nk + 2, :], st[:], Exp,
                                    scale=SCALE_EFF, bias=negshift[:])
                            else:
                                for j in range(2):
                                    nc.scalar.activation(
                                        ex[:, nk + j, :], st[:, j, :], Exp,
                                        scale=SCALE_EFF,
                                        bias=cb_sb[:, nk + j:nk + j + 1])
                        if kk == NKC - 4:
                            # softmax denominator before the last PV block:
                            # recip computes during the final PV chains, so
                            # the tail drain is just casts + out-proj.
                            den = st_ps.tile([P, 2, 512], f32, tag="st",
                                             name=f"den_{t}")
                            for k2 in range(NKC // 2):
                                nc.tensor.matmul(
                                    den[0:1, 0, :], ones2[:, 0:2, 0:1],
                                    ex[:, 2 * k2:2 * k2 + 2, :],
                                    start=(k2 == 0), stop=(k2 == NKC // 2 - 1),
                                    perf_mode=DR,
                                )
                            den_row = small.tile([1, 512], f32, tag="den_row",
                                                 bufs=2)
                            nc.scalar.copy(den_row[:], den[0:1, 0, :])
                        for k2 in range(kk // 2, kk // 2 + 2):
                            for u in range(C):
                                nc.tensor.matmul(
                                    ctx_psums[u][:],
                                    xn_sb[:, 2 * k2:2 * k2 + 2,
                                          u * P:(u + 1) * P],
                                    ex[:, 2 * k2:2 * k2 + 2, :],
                                    start=(k2 == 0), stop=(k2 == NKC // 2 - 1),
                                    perf_mode=DR,
                                )
                    for s in range(4):
                        # K=1 matmul == transpose of a 128-wide row slice
                        nc.tensor.matmul(
                            den[:, 1, s:s + 1], den_row[0:1, s * P:(s + 1) * P],
                            one_one[:], start=True, stop=True,
                        )
                    recip = small.tile([P, C], f32, tag="recip", bufs=2)
                    nc.vector.reciprocal(recip[:], den[:, 1, 0:4])

                    # ctx psum -> fp8, alternating engines so the tail isn't
                    # serialized on one of them
                    ctxT = ctxp.tile([P, C, 512], f8, tag="ctxT")
                    for u in range(C):
                        if u % 2 == 0:
                            nc.scalar.activation(
                                ctxT[:, u, :], ctx_psums[u][:], Copy,
                                scale=CTX_SCL)
                        else:
                            nc.vector.tensor_scalar_mul(
                                ctxT[:, u, :], ctx_psums[u][:], CTX_SCL)

                    for s in range(4):  # nq sub-chunks of 128
                        pj = ctx_ps.tile([P, 512], f32, tag="ctx",
                                         name=f"pj_{t}_{s}")
                        for c2 in range(2):
                            nc.tensor.matmul(
                                pj[:],
                                ctxT[:, 2 * c2:2 * c2 + 2, s * P:(s + 1) * P],
                                wp[:, 2 * c2:2 * c2 + 2, :],
                                start=(c2 == 0), stop=(c2 == 1),
                                perf_mode=DR,
                            )
                        r0 = t * 4 + s
                        o = iop.tile([P, U], bf16, tag="o")
                        # o = pj * recip (per-partition) + xq; the last tile
                        # computes in halves so the first half's writeback
                        # overlaps the second half's fusion
                        dma_eng = nc.sync if s % 2 == 0 else nc.scalar
                        if t == NQT - 1 and s >= 2:
                            for hh in range(2):
                                hs = slice(hh * (U // 2), (hh + 1) * (U // 2))
                                nc.vector.scalar_tensor_tensor(
                                    o[:, hs], pj[:, hs], recip[:, s:s + 1],
                                    xq_sb[:, r0, hs], Mult, Add,
                                )
                                dma_eng.dma_start(
                                    out_d[r0 * P:(r0 + 1) * P, hs], o[:, hs])
                        else:
                            nc.vector.scalar_tensor_tensor(
                                o[:], pj[:], recip[:, s:s + 1], xq_sb[:, r0, :],
                                Mult, Add,
                            )
                            dma_eng.dma_start(
                                out_d[r0 * P:(r0 + 1) * P, :], o[:])

    nc.compile()
    return nc


def _get_nc(has_c=False):
    key = ("nc", has_c)
    if key not in _CACHE:
        _CACHE[key] = _build_nc(has_c)
    return _CACHE[key]


def make_in_maps(x, Wq, bq, Wk, bk, Wv, bv, Wp, bp):
    x = np.asarray(x, np.float32)
    Wq = np.asarray(Wq, np.float32)
    bq = np.asarray(bq, np.float32)
    Wk = np.asarray(Wk, np.float32)
    bk = np.asarray(bk, np.float32)
    Wv = np.asarray(Wv, np.float32)
    bv = np.asarray(bv, np.float32)
    Wp = np.asarray(Wp, np.float32)
    bp = np.asarray(bp, np.float32)

    # folded weights: S = x M x^T with M = Wq Wk^T, folded into the Q side
    # (device computes q' = xq M over only its own NQ rows); out-proj
    # weight W2 = Wv Wp
    Wm = Wq @ Wk.T
    W2 = Wv @ Wp
    # attn rows sum to 1 => bv contributes bv @ Wp to every output row
    bconst = (bv @ Wp + bp).astype(np.float32)
    # nonzero bq adds the per-key column term c_j = x_j.(Wk bq) + bq.bk
    has_c = bool(np.any(bq != 0.0))

    import ml_dtypes
    f8 = ml_dtypes.float8_e4m3
    bf16 = ml_dtypes.bfloat16

    def q8(a):
        return np.ascontiguousarray(np.clip(a, -240.0, 240.0)).astype(f8)

    Wm8, W28 = q8(Wm * WS), q8(W2 * WS)

    in_maps = []
    for core in range(NCORES):
        b, h = core // 2, core % 2
        xb = np.ascontiguousarray(x[b])                       # [N, U]
        # keys ordered [own half, other half]; Q slice = first NQ columns
        perm = np.r_[h * NQ:(h + 1) * NQ, (1 - h) * NQ:(2 - h) * NQ]
        im = {
            "xT": q8(xb.T[:, perm]),
            "xn": q8(xb[perm]),
            "xq": np.ascontiguousarray(
                xb[h * NQ:(h + 1) * NQ] + bconst[None, :]).astype(bf16),
            "Wm": Wm8, "W2": W28,
        }
        if has_c:
            cvec = xb[perm] @ (Wk @ bq) + float(bq @ bk)
            im["cb"] = (cvec * 1.0 - SHIFT).astype(np.float32)
        in_maps.append(im)
    return in_maps, has_c


def gather_out(results):
    out = np.empty((B, N, U), np.float32)
    for core in range(NCORES):
        b, h = core // 2, core % 2
        out[b, h * NQ:(h + 1) * NQ] = np.asarray(
            results[core]["out"], dtype=np.float32)
    return out


def kernel(x, Wq, bq, Wk, bk, Wv, bv, Wp, bp):
    from concourse.bass_utils import run_bass_kernel_spmd

    in_maps, has_c = make_in_maps(x, Wq, bq, Wk, bk, Wv, bv, Wp, bp)
    nc = _get_nc(has_c)
    res = run_bass_kernel_spmd(nc, in_maps, core_ids=list(range(NCORES)))
    return gather_out(res.results)
